# revision 52
# baseline (speedup 1.0000x reference)
"""Trainium2 Bass kernel for nn_BinaryQuantumClassifier.

Math: the 4-qubit circuit collapses to a closed form. Per sample, with
theta_j = pi * (x @ W_ctq.T + b_ctq)_j  (j = 4r + i, reuse r, qubit i):
    d_i = a_i + R_i sin(pi * E_j),   E_j = (x @ W_ctq.T)_j + bs_j
and the CNOT chain maps Z-expectations to products of the d_i:
    z0 = d1 d2 d3, z1 = d0 d1, z2 = d0 d1 d2, z3 = d0 d1 d2 d3.
With d'_i = sin(pi E) + a_i/R_i the R_i factors fold into the final
per-k class weights, so out_c = sum_{k,r} (.5 W_cls[c,k] PR_k) z'_k +
b_cls[c] - the weights AND b_cls ride as tensor_scalar immediates.

Device plan per core (8192 samples = 64 groups of 128). HBM-bound
streaming x fp16 (~390 GB/s aggregate over both HWDGE queues, which
drain CONCURRENTLY at ~half rate each). Findings baked in from six
measured iterations:
  - 6 chunks [20, 16, 12, 8, 5, 3] groups; each chunk is one half-tile
    on EACH queue so it completes at the cumulative-bytes point of the
    combined stream (per-queue-contiguous tiles complete at 2x
    cumulative time and dump half the work past the stream end).
  - x is the PE's STATIONARY operand (lhsT [128 D x 128 samples]), rhs
    the fp16 W chunk [128 D x 8]; per-chunk phase-shift bias via one
    K=2 matmul of fp16 hi/lo rows (fp32-exact), accumulated in PSUM.
    No other PE work mid-stream.
  - Groups arrive LINEARLY in time and DVE/GpSimd ops cost 130-250ns
    FIXED each (GpSimd runs at 0.42 efficiency), so per-chunk work is
    3 ScalarE ACTs + 1 Vector op, and everything after sin runs on
    batches of two chunks:
      per chunk: t1 = Identity(E + 1.5*2^24); k2 = Identity(t1 -
      1.5*2^24)  (ScalarE fp32 round-to-even; ACT reads PSUM);
      r = E - k2 (Vector, the only V + ACT PSUM readers);
      s = Sin(pi r) = sin(pi E) (ScalarE) into the batch s-buffer.
      The two TAIL chunks instead do k2/r on Vector (the ScalarE
      pipeline t1->k2->r->sin serializes with earlier chunks at the
      drain - measured on v7).
      per batch: d' = s + aw'; v = d2 d3; z1 = d0 d1; z0 = d1 v;
      z2 = z1 d2; z3 = z1 v (fp16); Zw[c,g,k,r] = z_k wck' + b_c/8
      (8 tensor_scalar immediates - no const DMA, and the two XY
      tensor_reduce per class then read CONTIGUOUS [p,g,4,2]).
  - Batch plan: (c0,c1) and (c2,c3) split ops V/G mid-stream; the tail
    is (c4) all-GpSimd and (c5) all-Vector running in PARALLEL, with
    batch (c2,c3)'s Vector Zw/reduce emitted after the tail chunks'
    PSUM reads so the in-order Vector queue never blocks the drain.
  - Stores: batches 0-1 share one [128, 112] column store whose
    descriptor sits behind all x on the sync queue; the 16-col tail is
    PE-transposed (f32 identity, last on queue B) to [16, 128] so the
    final store is 16 row-packets instead of 128 column-packets.
Degenerate R_i ~ 0 switches to an unfolded build (extra batch multiply
off a misc Rw block that only exists in that build) via the consts key.
"""

import numpy as np

import concourse.bass as bass
import concourse.mybir as mybir
from concourse import bass_utils
from concourse.tile import TileContext

B, D, NQ = 65536, 512, 4
NCORES = 8
BC = B // NCORES            # 8192 samples per core
NCH = D // 128              # 4 K-chunks
NG = BC // 128              # 64 sample-groups per core (128 samples each)
GW = NCH * 128              # 512: x columns per sample-group

# chunks: (name, group_start, n_groups, groups_on_queue_A)
CHUNKS = [
    ("c0", 0, 20, 10), ("c1", 20, 16, 8),
    ("c2", 36, 12, 6), ("c3", 48, 8, 4),
    ("c4", 56, 5, 3), ("c5", 61, 3, 1),
]
# batches: (chunk indices, mode); 'vg' splits V/G, 'v'/'g' single-engine
BATCHES = [((0, 1), "vg"), ((2, 3), "vg2"), ((4,), "g"), ((5,), "v")]
N_O1 = 2                    # first N_O1 batches -> column store o1
WMAX = max(w for (_n, _g, w, _a) in CHUNKS)          # 20
CBW = 8 * WMAX                                       # 160: chunk block width
GBMAX = max(sum(CHUNKS[i][2] for i in ci) for (ci, _m) in BATCHES)
BBW = 8 * GBMAX                                      # 288: batch block width
PI = float(np.pi)
M2 = float(np.float32(1.5 * 2 ** 24))   # round-to-even-integer magic
MM_DT = mybir.dt.float16    # PE operand / const dtype
F32 = mybir.dt.float32
AL = mybir.AluOpType
AF = mybir.ActivationFunctionType
AX = mybir.AxisListType
# misc (queue A, fp16): wfa 32 | ones 128 | aw BBW | (Rw BBW, unfolded
# builds only)
MW_WFA, MW_ONES, MW_AW = 0, 32, 160
MW_RW = MW_AW + BBW
MW1 = MW_RW + BBW

_BG = []                                # (g0, gb) per batch
for (ci, _m) in BATCHES:
    _BG.append((CHUNKS[ci[0]][1], sum(CHUNKS[i][2] for i in ci)))
OCOLS = 2 * NG                          # 128
O1C = 2 * _BG[N_O1][0]                  # 112 cols -> o1
O2C = OCOLS - O1C                       # 16 cols -> transposed o2


def _split_waits(nc, max_waits=1):
    """walrus in this env accepts at most one sync-wait per instruction;
    move extras onto preceding same-engine NoOps."""
    for fn in nc.m.functions:
        for blk in fn.blocks:
            new_list = []
            for inst in blk.instructions:
                si = inst.sync_info
                if si is not None and len(si.on_wait) > max_waits:
                    waits = list(si.on_wait)
                    keep, extra = waits[-max_waits:], waits[:-max_waits]
                    for k, w in enumerate(extra):
                        new_list.append(mybir.InstNoOp(
                            name=f"{inst.name}-ws{k}", engine=inst.engine,
                            ins=[], outs=[],
                            sync_info=mybir.SyncInfo(on_wait=[w], on_update=[])))
                    si.on_wait = keep
                    inst.sync_info = si
                new_list.append(inst)
            blk.instructions = new_list


def _build_nc(consts):
    """consts: (bc2, fold, wck', ap') immediates; misc carries the rest."""
    bc2, fold, wck, _ap4 = consts
    mw = MW_RW if fold else MW1
    nc = bass.Bass("TRN2", target_bir_lowering=False)
    # x relayout: xa[p, g*512 + k*128 + ms] = x_core[128 g + ms, 128 k + p]
    xa_d = nc.dram_tensor("xa", [128, BC * NCH], MM_DT, kind="ExternalInput").ap()
    misc_d = nc.dram_tensor("misc", [128, mw], MM_DT, kind="ExternalInput").ap()
    bias_d = nc.dram_tensor("bias2", [2, CBW], MM_DT, kind="ExternalInput").ap()
    cf_d = nc.dram_tensor("cf", [128, 4], F32, kind="ExternalInput").ap()
    id_d = nc.dram_tensor("ident", [128, 128], F32, kind="ExternalInput").ap()
    o1_d = nc.dram_tensor("o1", [128, O1C], F32, kind="ExternalOutput").ap()
    o2_d = nc.dram_tensor("o2", [O2C, 128], F32, kind="ExternalOutput").ap()

    with TileContext(nc) as tc:
        with tc.tile_pool(name="wp", bufs=1) as wpool, \
             tc.tile_pool(name="xp", bufs=1) as xpool, \
             tc.tile_pool(name="pe", bufs=3, space="PSUM") as pspoolE, \
             tc.tile_pool(name="pt", bufs=1, space="PSUM") as pspoolT, \
             tc.tile_pool(name="ep", bufs=1) as epool:
            xts = {}

            def xtrig(nm, q, g0, w):
                if w == 0:
                    return
                eng = nc.sync if q == 0 else nc.scalar
                xt = xpool.tile([128, w * GW], MM_DT, name=f"x{nm}q{q}")
                eng.dma_start(xt[:], xa_d[:, g0 * GW:(g0 + w) * GW])
                xts[(nm, q)] = xt

            # queue A: cf+misc+bias lead (first bias matmul needs them);
            # queue B: x halves, f32 identity last (needed ~drain time)
            cf = wpool.tile([128, 4], F32, name="cf")
            nc.sync.dma_start(cf[:], cf_d[:])
            misc = wpool.tile([128, mw], MM_DT, name="misc")
            nc.sync.dma_start(misc[:], misc_d[:])
            bias2 = wpool.tile([2, CBW], MM_DT, name="bias2")
            nc.sync.dma_start(bias2[:], bias_d[:])
            for (nm, g0, w, wa) in CHUNKS:
                xtrig(nm, 0, g0, wa)
                xtrig(nm, 1, g0 + wa, w - wa)
            ident = wpool.tile([128, 128], F32, name="ident")
            nc.scalar.dma_start(ident[:], id_d[:])

            ones = misc[0:2, MW_ONES:MW_ONES + 128]
            O2a = epool.tile([128, O1C], F32, name="O2a")
            O2b = epool.tile([128, O2C], F32, name="O2b")
            sbufs = [epool.tile([128, 8 * gb], F32, name=f"sb{b}")
                     for b, (g0, gb) in enumerate(_BG)]

            def emit_mms(nm, g0, w, wa):
                W = 8 * w
                E = pspoolE.tile([128, CBW], F32, tag="E", name=f"E{nm}")
                nc.tensor.matmul(E[:, 0:W], ones, bias2[:, 0:W],
                                 start=True, stop=False, skip_group_check=True)
                for g in range(w):
                    q = 0 if g < wa else 1
                    xt = xts[(nm, q)]
                    gl = g if g < wa else g - wa
                    for k in range(NCH):
                        off = gl * GW + k * 128
                        nc.tensor.matmul(E[:, 8 * g:8 * g + 8],
                                         xt[:, off:off + 128],
                                         misc[:, MW_WFA + 8 * k:MW_WFA + 8 * k + 8],
                                         start=False, stop=(k == NCH - 1),
                                         skip_group_check=True)
                return E[:, 0:W]

            def emit_chunk_epi(b, nm, g0, w, E, on_v):
                """round-to-even + r; Sin lands in the batch s-buffer."""
                W = 8 * w
                r_ = epool.tile([128, W], F32, name=f"r{nm}")
                if on_v:
                    k2 = epool.tile([128, W], F32, name=f"k2{nm}")
                    nc.vector.tensor_scalar(k2[:], E[:], M2, M2,
                                            AL.add, AL.subtract)
                    nc.vector.tensor_sub(r_[:], E[:], k2[:])
                else:
                    t1 = epool.tile([128, W], F32, name=f"t1{nm}")
                    k2 = epool.tile([128, W], F32, name=f"k2{nm}")
                    nc.scalar.activation(t1[:], E[:], AF.Identity,
                                         bias=cf[:, 0:1])
                    nc.scalar.activation(k2[:], t1[:], AF.Identity,
                                         bias=cf[:, 1:2])
                    nc.vector.tensor_sub(r_[:], E[:], k2[:])
                so = 8 * (g0 - _BG[b][0])
                nc.scalar.activation(sbufs[b][:, so:so + W], r_[:],
                                     AF.Sin, scale=PI)

            def batch_tiles(b):
                g0, gb = _BG[b]
                W = 8 * gb
                d_ = epool.tile([128, W], MM_DT, name=f"d{b}")
                v_ = epool.tile([128, 2 * gb], MM_DT, name=f"v{b}")
                z_ = epool.tile([128, W], MM_DT, name=f"z{b}")
                Zw = epool.tile([128, 2, gb, 4, 2], MM_DT, name=f"Zw{b}")
                return d_, v_, z_, Zw

            def emit_products(b, tiles, eD, eP0, eP1):
                g0, gb = _BG[b]
                W = 8 * gb
                d_, v_, z_, Zw = tiles
                s_ = sbufs[b]
                if fold:
                    eD.tensor_add(d_[:], s_[:], misc[:, MW_AW:MW_AW + W])
                else:
                    t_ = epool.tile([128, W], F32, name=f"t{b}")
                    eD.tensor_mul(t_[:], s_[:], misc[:, MW_RW:MW_RW + W])
                    eD.tensor_add(d_[:], t_[:], misc[:, MW_AW:MW_AW + W])
                d4 = d_.rearrange("p (u q) -> p q u", q=4)

                def zk(k):
                    return z_[:, 2 * gb * k:2 * gb * (k + 1)]

                eP1.tensor_mul(v_[:], d4[:, 2, :], d4[:, 3, :])   # v = d2 d3
                eP0.tensor_mul(zk(1), d4[:, 0, :], d4[:, 1, :])   # z1 = d0 d1
                eP1.tensor_mul(zk(0), d4[:, 1, :], v_[:])         # z0 = d1 v
                eP0.tensor_mul(zk(2), zk(1), d4[:, 2, :])         # z2 = z1 d2
                eP1.tensor_mul(zk(3), zk(1), v_[:])               # z3 = z1 v

            def emit_zw(b, tiles, engs):
                g0, gb = _BG[b]
                _d, _v, z_, Zw = tiles
                for c in range(2):
                    for k in range(4):
                        zv = z_[:, 2 * gb * k:2 * gb * (k + 1)].rearrange(
                            "p (g r) -> p g r", r=2)
                        engs[(c * 4 + k) % len(engs)].tensor_scalar(
                            Zw[:, c, :, k, :], zv,
                            float(wck[c][k]), float(bc2[c] / 8.0),
                            AL.mult, AL.add)

            def emit_reduces(b, tiles):
                g0, gb = _BG[b]
                Zw = tiles[3]
                co = 2 * g0
                Ot, cb = (O2a, co) if co < O1C else (O2b, co - O1C)
                for c in range(2):
                    nc.vector.tensor_reduce(
                        Ot[:, cb + c * gb:cb + (c + 1) * gb],
                        Zw[:, c], AX.XY, AL.add)

            V, G = nc.vector, nc.gpsimd
            # --- batch 0 (c0+c1): steady-state, ops split V/G ---
            for ci in BATCHES[0][0]:
                (nm, g0, w, wa) = CHUNKS[ci]
                emit_chunk_epi(0, nm, g0, w, emit_mms(nm, g0, w, wa), False)
            t0 = batch_tiles(0)
            emit_products(0, t0, G, V, G)
            emit_zw(0, t0, [V, G])
            emit_reduces(0, t0)
            # --- batch 1 (c2+c3): products now, Zw/reduce deferred ---
            for ci in BATCHES[1][0]:
                (nm, g0, w, wa) = CHUNKS[ci]
                emit_chunk_epi(1, nm, g0, w, emit_mms(nm, g0, w, wa), False)
            t1_ = batch_tiles(1)
            emit_products(1, t1_, G, V, G)
            # --- tail chunks: matmuls + Vector PSUM reads first ---
            tailE = {}
            for (bi, _m) in ((2, "g"), (3, "v")):
                (nm, g0, w, wa) = CHUNKS[BATCHES[bi][0][0]]
                tailE[bi] = emit_mms(nm, g0, w, wa)
            for (bi, _m) in ((2, "g"), (3, "v")):
                (nm, g0, w, wa) = CHUNKS[BATCHES[bi][0][0]]
                emit_chunk_epi(bi, nm, g0, w, tailE[bi], True)
            # --- batch 1 finish (all-Vector) + big store ---
            emit_zw(1, t1_, [V])
            emit_reduces(1, t1_)
            nc.sync.dma_start(o1_d[:], O2a[:])
            # --- tail batches: c5 all-Vector, c4 all-GpSimd (parallel) ---
            t3 = batch_tiles(3)
            emit_products(3, t3, V, V, V)
            emit_zw(3, t3, [V])
            t2 = batch_tiles(2)
            emit_products(2, t2, G, G, G)
            emit_zw(2, t2, [G])
            emit_reduces(3, t3)
            emit_reduces(2, t2)

            # tail store: PE transpose so it is O2C row-packets
            pT = pspoolT.tile([128, 128], F32, name="pT")
            nc.tensor.transpose(pT[0:O2C, 0:128], O2b[:], ident[:])
            oT = epool.tile([O2C, 128], F32, name="oT")
            nc.vector.tensor_copy(oT[:], pT[0:O2C, 0:128])
            nc.scalar.dma_start(o2_d[:], oT[:])

    return nc


_NC_CACHE = {}


def _get_nc(consts, split=True):
    key = ("nc8", split, consts)
    if key not in _NC_CACHE:
        nc = _build_nc(consts)
        if split:
            _split_waits(nc)
        _NC_CACHE[key] = nc
    return _NC_CACHE[key]


def _qubit_abc(q_params):
    """Exact (a_i, b_i, c_i) with d_i(theta) = a + b sin(theta) + c cos(theta)."""
    out = np.zeros((NQ, 3), np.float64)
    for i in range(NQ):
        pa, pb, pc = [float(v) for v in q_params[3 * i:3 * i + 3]]

        def rx(t):
            return np.array([[np.cos(t / 2), -1j * np.sin(t / 2)],
                             [-1j * np.sin(t / 2), np.cos(t / 2)]])

        def ry(t):
            return np.array([[np.cos(t / 2), -np.sin(t / 2)],
                             [np.sin(t / 2), np.cos(t / 2)]])

        def rz(t):
            return np.array([[np.exp(-0.5j * t), 0], [0, np.exp(0.5j * t)]])

        H = np.array([[1, 1], [1, -1]]) / np.sqrt(2)
        U = rz(pc) @ ry(pb) @ rx(pa)

        def dfun(theta):
            v = U @ ry(theta) @ H @ np.array([1.0, 0.0])
            pr = np.abs(v) ** 2
            return pr[0] - pr[1]

        d0, dpi, dh = dfun(0.0), dfun(np.pi), dfun(np.pi / 2)
        a = (d0 + dpi) / 2
        c = (d0 - dpi) / 2
        b = dh - a
        out[i] = (a, b, c)
    return out


def _make_consts(b_ctq, q_params, W_cls, b_cls):
    abc = _qubit_abc(q_params)
    R4, a4, bs = np.zeros(4), np.zeros(4), np.zeros(8)
    for i in range(4):
        a, b, c_ = abc[i]
        R4[i] = np.hypot(b, c_)
        a4[i] = a
    for j in range(8):
        _, b, c_ = abc[j % 4]
        bs[j] = b_ctq[j] + np.arctan2(c_, b) / np.pi
    fold = bool(np.min(R4) > 1e-3)
    bc2 = tuple(float(np.float32(v)) for v in b_cls)

    if fold:
        ap = a4 / R4
        RP = np.array([R4[1] * R4[2] * R4[3], R4[0] * R4[1],
                       R4[0] * R4[1] * R4[2], R4[0] * R4[1] * R4[2] * R4[3]])
    else:
        ap = a4
        RP = np.ones(4)
    wp = 0.5 * np.asarray(W_cls, np.float64)
    wck = tuple(tuple(float(np.float32(wp[c, k] * RP[k])) for k in range(4))
                for c in range(2))
    consts = (bc2, fold, wck,
              tuple(float(np.float32(v)) for v in ap))

    mw = MW_RW if fold else MW1
    misc = np.zeros((128, mw), np.float16)
    misc[:, MW_ONES:MW_ONES + 128] = 1.0
    misc[:, MW_AW:MW_RW] = np.tile(ap, BBW // 4).astype(np.float16)
    if not fold:
        misc[:, MW_RW:MW1] = np.tile(R4, BBW // 4).astype(np.float16)
    # bias rows: row0 = fp16 hi, row1 = residual lo (hi+lo == fp32 bs)
    bias2 = np.zeros((2, CBW), np.float16)
    bs_t = np.tile(bs, CBW // 8)
    bhi = bs_t.astype(np.float16)
    bias2[0, :] = bhi
    bias2[1, :] = (bs_t - bhi.astype(np.float64)).astype(np.float16)
    cf = np.zeros((128, 4), np.float32)
    cf[:, 0] = M2
    cf[:, 1] = -M2
    return consts, misc, bias2, cf


def make_in_maps(x, W_ctq, b_ctq, q_params, W_cls, b_cls):
    consts, misc, bias2, cf = _make_consts(
        np.asarray(b_ctq, np.float32), np.asarray(q_params, np.float32),
        np.asarray(W_cls, np.float32), np.asarray(b_cls, np.float32))
    wt = np.asarray(W_ctq, np.float32).T                        # [512, 8]
    misc[:, MW_WFA:MW_WFA + 32] = \
        wt.reshape(NCH, 128, 8).transpose(1, 0, 2).reshape(128, 32)
    misc = np.ascontiguousarray(misc)
    ident = np.eye(128, dtype=np.float32)
    x = np.asarray(x, np.float32)
    in_maps = []
    for c in range(NCORES):
        xs = x[c * BC:(c + 1) * BC]                             # [8192, 512]
        # relayout: [p, g*512 + k*128 + ms] = xs[128 g + ms, 128 k + p]
        xa = np.ascontiguousarray(
            xs.reshape(NG, 128, NCH, 128).transpose(3, 0, 2, 1)
              .reshape(128, BC * NCH)).astype(np.float16)
        in_maps.append({"xa": xa, "misc": misc, "bias2": bias2, "cf": cf,
                        "ident": ident})
    return in_maps, consts


def assemble_output(results):
    out = np.empty((B, 2), np.float32)
    for core in range(NCORES):
        o1 = results[core]["o1"]                                 # [128, O1C]
        o2 = results[core]["o2"]                                 # [O2C, 128]
        for b, (g0, gb) in enumerate(_BG):
            co = 2 * g0
            for c in range(2):
                if co < O1C:
                    blk = o1[:, co + c * gb:co + (c + 1) * gb]   # [128, gb]
                else:
                    blk = o2[co - O1C + c * gb:co - O1C + (c + 1) * gb, :].T
                # blk[p, g] = out_c(sample 128 (g0+g) + p)
                out[core * BC + 128 * g0:core * BC + 128 * (g0 + gb), c] = \
                    blk.T.reshape(-1)
    return out


def kernel(x, W_ctq, b_ctq, q_params, W_cls, b_cls):
    in_maps, consts = make_in_maps(x, W_ctq, b_ctq, q_params, W_cls, b_cls)
    nc = _get_nc(consts)
    res = bass_utils.run_bass_kernel_spmd(nc, in_maps, core_ids=list(range(NCORES)))
    return assemble_output(res.results)


# revision 54
# speedup vs baseline: 1.0078x; 1.0078x over previous
"""Trainium2 Bass kernel for nn_BinaryQuantumClassifier.

Math: the 4-qubit circuit collapses to a closed form. Per sample, with
theta_j = pi * (x @ W_ctq.T + b_ctq)_j  (j = 4r + i, reuse r, qubit i):
    d_i = a_i + R_i sin(pi * E_j),   E_j = (x @ W_ctq.T)_j + bs_j
and the CNOT chain maps Z-expectations to products of the d_i:
    z0 = d1 d2 d3, z1 = d0 d1, z2 = d0 d1 d2, z3 = d0 d1 d2 d3.
With d'_i = sin(pi E) + a_i/R_i the R_i factors fold into the final
per-k class weights, so out_c = sum_{k,r} (.5 W_cls[c,k] PR_k) z'_k +
b_cls[c] - weights, b_cls AND the a_i/R_i offsets all ride as
tensor_scalar immediates (no wide constant tensors).

Device plan per core (8192 samples = 64 groups of 128). HBM-bound
streaming x fp16 (~390 GB/s aggregate over both HWDGE queues). Key
measured facts this version encodes:
  - DMA transfers pay a PER-PARTITION-ROW packet cost: a [128, 448]
    fp16 const tile measured 6.3us for 114KB and stalled the queue, so
    v9 keeps only wfa [128, 32] + cf [128, 4] as 128-row consts; the
    K=2 bias matmul operands ride ONE [2, 288] 2-packet tensor
    (ones || fp16 hi/lo phase rows), and aw/wcs/b_cls are immediates.
  - 6 chunks [20, 16, 12, 8, 5, 3]; each chunk is one half-tile per
    queue so it completes at the cumulative-bytes point of the stream.
  - x is the PE's STATIONARY operand (lhsT [128 D x 128 samples]), rhs
    the fp16 W chunk [128 D x 8]; bias via one K=2 matmul (fp32-exact
    hi/lo), accumulated in PSUM. No other PE work mid-stream.
  - DVE/GpSimd ops cost 130-330ns FIXED each, so per-chunk work is
    tiny and the post-sin stages run on batches of two chunks:
      per chunk: t1 = Identity(E + 1.5*2^24), k2 = Identity(t1 -
      1.5*2^24) (ScalarE fp32 round-to-even, reads PSUM), r = E - k2
      (Vector), s = Sin(pi r) (ScalarE) -> batch s-buffer. Tail chunks
      do k2 on Vector instead (the ScalarE pipeline serializes at the
      drain - measured).
      per batch: d'_i = s_i + a'_i (4 stride-4 TS), products (5 TT,
      fp16), Zw[c,g,k,r] = z_k wck' + b_c/8 (8 TS), 2 contiguous XY
      tensor_reduce -> O2 column block. Batches 0/1 split V/G; the
      tail batch (c4+c5) runs on Vector with d'2/d'3/v lent to GpSimd,
      and batch1's deferred V-tail sits after the tail PSUM reads so
      the in-order Vector queue never blocks the drain.
  - Stores: batches 0/1 share one [128, 112] column store (sync queue,
    behind all x); the 16-col tail batch is PE-transposed (f32
    identity rides the gap after x on queue A) into 16 row-packets.
Degenerate R_i ~ 0 falls back to an unfolded build (wfa-width Rw
multiply via immediates is impossible, so it keeps a [128, BBW] Rw
const) via the consts cache key.
"""

import numpy as np

import concourse.bass as bass
import concourse.mybir as mybir
from concourse import bass_utils
from concourse.tile import TileContext

B, D, NQ = 65536, 512, 4
NCORES = 8
BC = B // NCORES            # 8192 samples per core
NCH = D // 128              # 4 K-chunks
NG = BC // 128              # 64 sample-groups per core (128 samples each)
GW = NCH * 128              # 512: x columns per sample-group

# chunks: (name, group_start, n_groups, groups_on_queue_A)
CHUNKS = [
    ("c0", 0, 20, 10), ("c1", 20, 16, 8),
    ("c2", 36, 12, 6), ("c3", 48, 8, 4),
    ("c4", 56, 5, 2), ("c5", 61, 3, 1),
]
# batches: (chunk indices, mode)
BATCHES = [((0, 1), "vg"), ((2, 3), "vg"), ((4, 5), "v")]
N_O1 = 2                    # first N_O1 batches -> column store o1
WMAX = max(w for (_n, _g, w, _a) in CHUNKS)          # 20
CBW = 8 * WMAX                                       # 160: chunk block width
GBMAX = max(sum(CHUNKS[i][2] for i in ci) for (ci, _m) in BATCHES)
BBW = 8 * GBMAX                                      # 288: batch block width
PI = float(np.pi)
M2 = float(np.float32(1.5 * 2 ** 24))   # round-to-even-integer magic
MM_DT = mybir.dt.float16    # PE operand / const dtype
F32 = mybir.dt.float32
AL = mybir.AluOpType
AF = mybir.ActivationFunctionType
AX = mybir.AxisListType

_BG = []                                # (g0, gb) per batch
for (ci, _m) in BATCHES:
    _BG.append((CHUNKS[ci[0]][1], sum(CHUNKS[i][2] for i in ci)))
OCOLS = 2 * NG                          # 128
O1C = 2 * _BG[N_O1][0]                  # 112 cols -> o1
O2C = OCOLS - O1C                       # 16 cols -> transposed o2


def _split_waits(nc, max_waits=1):
    """walrus in this env accepts at most one sync-wait per instruction;
    move extras onto preceding same-engine NoOps."""
    for fn in nc.m.functions:
        for blk in fn.blocks:
            new_list = []
            for inst in blk.instructions:
                si = inst.sync_info
                if si is not None and len(si.on_wait) > max_waits:
                    waits = list(si.on_wait)
                    keep, extra = waits[-max_waits:], waits[:-max_waits]
                    for k, w in enumerate(extra):
                        new_list.append(mybir.InstNoOp(
                            name=f"{inst.name}-ws{k}", engine=inst.engine,
                            ins=[], outs=[],
                            sync_info=mybir.SyncInfo(on_wait=[w], on_update=[])))
                    si.on_wait = keep
                    inst.sync_info = si
                new_list.append(inst)
            blk.instructions = new_list


def _build_nc(consts):
    """consts: (bc2, fold, wck, ap4) immediates."""
    bc2, fold, wck, ap4 = consts
    nc = bass.Bass("TRN2", target_bir_lowering=False)
    # x relayout: xa[p, g*512 + k*128 + ms] = x_core[128 g + ms, 128 k + p]
    xa_d = nc.dram_tensor("xa", [128, BC * NCH], MM_DT, kind="ExternalInput").ap()
    wfa_d = nc.dram_tensor("wfa", [128, 32], MM_DT, kind="ExternalInput").ap()
    # ob rows: [ones(128) || bias hi/lo(CBW)]
    ob_d = nc.dram_tensor("ob", [2, 128 + CBW], MM_DT, kind="ExternalInput").ap()
    cf_d = nc.dram_tensor("cf", [128, 4], F32, kind="ExternalInput").ap()
    id_d = nc.dram_tensor("ident", [128, 128], F32, kind="ExternalInput").ap()
    rw_d = (nc.dram_tensor("rw", [128, BBW], MM_DT, kind="ExternalInput").ap()
            if not fold else None)
    o1_d = nc.dram_tensor("o1", [128, O1C], F32, kind="ExternalOutput").ap()
    o2_d = nc.dram_tensor("o2", [O2C, 128], F32, kind="ExternalOutput").ap()

    with TileContext(nc) as tc:
        with tc.tile_pool(name="wp", bufs=1) as wpool, \
             tc.tile_pool(name="xp", bufs=1) as xpool, \
             tc.tile_pool(name="pe", bufs=3, space="PSUM") as pspoolE, \
             tc.tile_pool(name="pt", bufs=1, space="PSUM") as pspoolT, \
             tc.tile_pool(name="ep", bufs=1) as epool:
            xts = {}

            def xtrig(nm, q, g0, w):
                if w == 0:
                    return
                eng = nc.sync if q == 0 else nc.scalar
                xt = xpool.tile([128, w * GW], MM_DT, name=f"x{nm}q{q}")
                eng.dma_start(xt[:], xa_d[:, g0 * GW:(g0 + w) * GW])
                xts[(nm, q)] = xt

            # queue A: cf/ob/wfa lead (tiny + one slow 32-col tile), x
            # halves, then identity + the o1 store fill the post-x gap;
            # queue B: x halves then the o2 store.
            cf = wpool.tile([128, 4], F32, name="cf")
            nc.sync.dma_start(cf[:], cf_d[:])
            ob = wpool.tile([2, 128 + CBW], MM_DT, name="ob")
            nc.sync.dma_start(ob[:], ob_d[:])
            wfa = wpool.tile([128, 32], MM_DT, name="wfa")
            nc.sync.dma_start(wfa[:], wfa_d[:])
            rw = None
            if not fold:
                rw = wpool.tile([128, BBW], MM_DT, name="rw")
                nc.scalar.dma_start(rw[:], rw_d[:])
            for (nm, g0, w, wa) in CHUNKS:
                xtrig(nm, 0, g0, wa)
                xtrig(nm, 1, g0 + wa, w - wa)
            ident = wpool.tile([128, 128], F32, name="ident")
            nc.sync.dma_start(ident[:], id_d[:])

            ones = ob[:, 0:128]
            O2a = epool.tile([128, O1C], F32, name="O2a")
            O2b = epool.tile([128, O2C], F32, name="O2b")
            sbufs = [epool.tile([128, 8 * gb], F32, name=f"sb{b}")
                     for b, (g0, gb) in enumerate(_BG)]

            def emit_mms(nm, g0, w, wa):
                W = 8 * w
                E = pspoolE.tile([128, CBW], F32, tag="E", name=f"E{nm}")
                nc.tensor.matmul(E[:, 0:W], ones, ob[:, 128:128 + W],
                                 start=True, stop=False, skip_group_check=True)
                for g in range(w):
                    q = 0 if g < wa else 1
                    xt = xts[(nm, q)]
                    gl = g if g < wa else g - wa
                    for k in range(NCH):
                        off = gl * GW + k * 128
                        nc.tensor.matmul(E[:, 8 * g:8 * g + 8],
                                         xt[:, off:off + 128],
                                         wfa[:, 8 * k:8 * k + 8],
                                         start=False, stop=(k == NCH - 1),
                                         skip_group_check=True)
                return E[:, 0:W]

            def emit_chunk_epi(b, nm, g0, w, E, on_v):
                W = 8 * w
                r_ = epool.tile([128, W], F32, name=f"r{nm}")
                k2 = epool.tile([128, W], F32, name=f"k2{nm}")
                if on_v:
                    nc.vector.tensor_scalar(k2[:], E[:], M2, M2,
                                            AL.add, AL.subtract)
                else:
                    t1 = epool.tile([128, W], F32, name=f"t1{nm}")
                    nc.scalar.activation(t1[:], E[:], AF.Identity,
                                         bias=cf[:, 0:1])
                    nc.scalar.activation(k2[:], t1[:], AF.Identity,
                                         bias=cf[:, 1:2])
                nc.vector.tensor_sub(r_[:], E[:], k2[:])
                so = 8 * (g0 - _BG[b][0])
                nc.scalar.activation(sbufs[b][:, so:so + W], r_[:],
                                     AF.Sin, scale=PI)

            def batch_tiles(b):
                g0, gb = _BG[b]
                W = 8 * gb
                d_ = epool.tile([128, W], MM_DT, name=f"d{b}")
                v_ = epool.tile([128, 2 * gb], MM_DT, name=f"v{b}")
                z_ = epool.tile([128, W], MM_DT, name=f"z{b}")
                Zw = epool.tile([128, 2, gb, 4, 2], MM_DT, name=f"Zw{b}")
                return d_, v_, z_, Zw

            def emit_dprime(b, tiles, engs):
                """d'_i = s_i + a'_i on stride-4 views (immediates); the
                unfolded fallback multiplies by an Rw const row first."""
                g0, gb = _BG[b]
                W = 8 * gb
                d_, _v, _z, _Zw = tiles
                s_ = sbufs[b]
                if not fold:
                    t_ = epool.tile([128, W], F32, name=f"t{b}")
                    engs[0].tensor_mul(t_[:], s_[:], rw[:, 0:W])
                    s_ = t_
                s4 = s_.rearrange("p (u q) -> p q u", q=4)
                d4 = d_.rearrange("p (u q) -> p q u", q=4)
                for i in range(4):
                    engs[i % len(engs)].tensor_scalar(
                        d4[:, i, :], s4[:, i, :], 1.0, float(ap4[i]),
                        AL.mult, AL.add)
                return d4

            def emit_products(b, tiles, d4, eP0, eP1):
                g0, gb = _BG[b]
                _d, v_, z_, _Zw = tiles

                def zk(k):
                    return z_[:, 2 * gb * k:2 * gb * (k + 1)]

                eP1.tensor_mul(v_[:], d4[:, 2, :], d4[:, 3, :])   # v = d2 d3
                eP0.tensor_mul(zk(1), d4[:, 0, :], d4[:, 1, :])   # z1 = d0 d1
                eP1.tensor_mul(zk(0), d4[:, 1, :], v_[:])         # z0 = d1 v
                eP0.tensor_mul(zk(2), zk(1), d4[:, 2, :])         # z2 = z1 d2
                eP1.tensor_mul(zk(3), zk(1), v_[:])               # z3 = z1 v

            def emit_zw(b, tiles, engs, cks=None):
                g0, gb = _BG[b]
                _d, _v, z_, Zw = tiles
                for n, (c, k) in enumerate(cks if cks is not None else
                                           [(c, k) for c in range(2)
                                            for k in range(4)]):
                    zv = z_[:, 2 * gb * k:2 * gb * (k + 1)].rearrange(
                        "p (g r) -> p g r", r=2)
                    engs[n % len(engs)].tensor_scalar(
                        Zw[:, c, :, k, :], zv,
                        float(wck[c][k]), float(bc2[c] / 8.0),
                        AL.mult, AL.add)

            def emit_reduces(b, tiles):
                g0, gb = _BG[b]
                Zw = tiles[3]
                co = 2 * g0
                Ot, cb = (O2a, co) if co < O1C else (O2b, co - O1C)
                for c in range(2):
                    nc.vector.tensor_reduce(
                        Ot[:, cb + c * gb:cb + (c + 1) * gb],
                        Zw[:, c], AX.XY, AL.add)

            V, G = nc.vector, nc.gpsimd
            # --- batch 0 (c0+c1): steady state, split V/G ---
            for ci in BATCHES[0][0]:
                (nm, g0, w, wa) = CHUNKS[ci]
                emit_chunk_epi(0, nm, g0, w, emit_mms(nm, g0, w, wa), False)
            t0 = batch_tiles(0)
            d40 = emit_dprime(0, t0, [V, V, G, G])
            emit_products(0, t0, d40, V, G)
            emit_zw(0, t0, [V, G])
            emit_reduces(0, t0)
            # --- batch 1 (c2+c3): V-part after the tail PSUM reads ---
            for ci in BATCHES[1][0]:
                (nm, g0, w, wa) = CHUNKS[ci]
                emit_chunk_epi(1, nm, g0, w, emit_mms(nm, g0, w, wa), False)
            t1_ = batch_tiles(1)
            d41 = emit_dprime(1, t1_, [V, V, G, G])
            emit_products(1, t1_, d41, V, G)
            # --- tail chunks c4/c5: matmuls + Vector k2/r + Sin ---
            tE = []
            for ci in BATCHES[2][0]:
                (nm, g0, w, wa) = CHUNKS[ci]
                tE.append((nm, g0, w, emit_mms(nm, g0, w, wa)))
            for (nm, g0, w, E) in tE:
                emit_chunk_epi(2, nm, g0, w, E, True)
            # --- batch 1 finish (Vector) + big store ---
            emit_zw(1, t1_, [V, G])
            emit_reduces(1, t1_)
            nc.sync.dma_start(o1_d[:], O2a[:])
            # --- tail batch (c4+c5): Vector chain, d'2/d'3/v on GpSimd ---
            t2 = batch_tiles(2)
            d42 = emit_dprime(2, t2, [V, V, G, G])
            emit_products(2, t2, d42, V, V)
            emit_zw(2, t2, [V])
            emit_reduces(2, t2)

            # tail store: PE transpose so it is O2C row-packets
            pT = pspoolT.tile([128, 128], F32, name="pT")
            nc.tensor.transpose(pT[0:O2C, 0:128], O2b[:], ident[:])
            oT = epool.tile([O2C, 128], F32, name="oT")
            nc.vector.tensor_copy(oT[:], pT[0:O2C, 0:128])
            nc.scalar.dma_start(o2_d[:], oT[:])

    return nc


_NC_CACHE = {}


def _get_nc(consts, split=True):
    key = ("nc9", split, consts)
    if key not in _NC_CACHE:
        nc = _build_nc(consts)
        if split:
            _split_waits(nc)
        _NC_CACHE[key] = nc
    return _NC_CACHE[key]


def _qubit_abc(q_params):
    """Exact (a_i, b_i, c_i) with d_i(theta) = a + b sin(theta) + c cos(theta)."""
    out = np.zeros((NQ, 3), np.float64)
    for i in range(NQ):
        pa, pb, pc = [float(v) for v in q_params[3 * i:3 * i + 3]]

        def rx(t):
            return np.array([[np.cos(t / 2), -1j * np.sin(t / 2)],
                             [-1j * np.sin(t / 2), np.cos(t / 2)]])

        def ry(t):
            return np.array([[np.cos(t / 2), -np.sin(t / 2)],
                             [np.sin(t / 2), np.cos(t / 2)]])

        def rz(t):
            return np.array([[np.exp(-0.5j * t), 0], [0, np.exp(0.5j * t)]])

        H = np.array([[1, 1], [1, -1]]) / np.sqrt(2)
        U = rz(pc) @ ry(pb) @ rx(pa)

        def dfun(theta):
            v = U @ ry(theta) @ H @ np.array([1.0, 0.0])
            pr = np.abs(v) ** 2
            return pr[0] - pr[1]

        d0, dpi, dh = dfun(0.0), dfun(np.pi), dfun(np.pi / 2)
        a = (d0 + dpi) / 2
        c = (d0 - dpi) / 2
        b = dh - a
        out[i] = (a, b, c)
    return out


def _make_consts(b_ctq, q_params, W_cls, b_cls):
    abc = _qubit_abc(q_params)
    R4, a4, bs = np.zeros(4), np.zeros(4), np.zeros(8)
    for i in range(4):
        a, b, c_ = abc[i]
        R4[i] = np.hypot(b, c_)
        a4[i] = a
    for j in range(8):
        _, b, c_ = abc[j % 4]
        bs[j] = b_ctq[j] + np.arctan2(c_, b) / np.pi
    fold = bool(np.min(R4) > 1e-3)
    bc2 = tuple(float(np.float32(v)) for v in b_cls)

    if fold:
        ap = a4 / R4
        RP = np.array([R4[1] * R4[2] * R4[3], R4[0] * R4[1],
                       R4[0] * R4[1] * R4[2], R4[0] * R4[1] * R4[2] * R4[3]])
    else:
        ap = a4
        RP = np.ones(4)
    wp = 0.5 * np.asarray(W_cls, np.float64)
    wck = tuple(tuple(float(np.float32(wp[c, k] * RP[k])) for k in range(4))
                for c in range(2))
    consts = (bc2, fold, wck, tuple(float(np.float32(v)) for v in ap))

    ob = np.zeros((2, 128 + CBW), np.float16)
    ob[:, 0:128] = 1.0
    bs_t = np.tile(bs, CBW // 8)
    bhi = bs_t.astype(np.float16)
    ob[0, 128:] = bhi
    ob[1, 128:] = (bs_t - bhi.astype(np.float64)).astype(np.float16)
    cf = np.zeros((128, 4), np.float32)
    cf[:, 0] = M2
    cf[:, 1] = -M2
    rw = np.tile(R4, BBW // 4).astype(np.float16) * np.ones((128, 1), np.float16)
    return consts, ob, cf, rw


def make_in_maps(x, W_ctq, b_ctq, q_params, W_cls, b_cls):
    consts, ob, cf, rw = _make_consts(
        np.asarray(b_ctq, np.float32), np.asarray(q_params, np.float32),
        np.asarray(W_cls, np.float32), np.asarray(b_cls, np.float32))
    (bc2, fold, wck, ap4) = consts
    wt = np.asarray(W_ctq, np.float32).T                        # [512, 8]
    wfa = np.ascontiguousarray(
        wt.reshape(NCH, 128, 8).transpose(1, 0, 2).reshape(128, 32)
    ).astype(np.float16)
    ident = np.eye(128, dtype=np.float32)
    x = np.asarray(x, np.float32)
    in_maps = []
    for c in range(NCORES):
        xs = x[c * BC:(c + 1) * BC]                             # [8192, 512]
        # relayout: [p, g*512 + k*128 + ms] = xs[128 g + ms, 128 k + p]
        xa = np.ascontiguousarray(
            xs.reshape(NG, 128, NCH, 128).transpose(3, 0, 2, 1)
              .reshape(128, BC * NCH)).astype(np.float16)
        im = {"xa": xa, "wfa": wfa, "ob": ob, "cf": cf, "ident": ident}
        if not fold:
            im["rw"] = np.ascontiguousarray(rw)
        in_maps.append(im)
    return in_maps, consts


def assemble_output(results):
    out = np.empty((B, 2), np.float32)
    for core in range(NCORES):
        o1 = results[core]["o1"]                                 # [128, O1C]
        o2 = results[core]["o2"]                                 # [O2C, 128]
        for b, (g0, gb) in enumerate(_BG):
            co = 2 * g0
            for c in range(2):
                if co < O1C:
                    blk = o1[:, co + c * gb:co + (c + 1) * gb]   # [128, gb]
                else:
                    blk = o2[co - O1C + c * gb:co - O1C + (c + 1) * gb, :].T
                # blk[p, g] = out_c(sample 128 (g0+g) + p)
                out[core * BC + 128 * g0:core * BC + 128 * (g0 + gb), c] = \
                    blk.T.reshape(-1)
    return out


def kernel(x, W_ctq, b_ctq, q_params, W_cls, b_cls):
    in_maps, consts = make_in_maps(x, W_ctq, b_ctq, q_params, W_cls, b_cls)
    nc = _get_nc(consts)
    res = bass_utils.run_bass_kernel_spmd(nc, in_maps, core_ids=list(range(NCORES)))
    return assemble_output(res.results)
